# revision 21
# baseline (speedup 1.0000x reference)
"""Causal single-head attention (B=4, N=2048, E=1024, D=64) on 8 TRN2 NeuronCores.

Sharding: core i handles batch b = i//2, query rows with parity p = i%2
(rows p, p+2, ...). The row-interleaved split makes the causal workload
identical on every core, so one SPMD program serves all 8. K/V are loaded in
full per core (no collectives); Q is the strided half.

The kernel streams K/V in 8 strips of 256 keys so that projections, scores,
softmax and AV all pipeline behind the DMA stream. Inputs stream in fp16
(halves HBM traffic; inputs are O(1)-scaled so fp16 rounding costs ~5e-4
relative error); on-chip score/AV matmuls run in fp32r at full PE rate with
fp32 PSUM accumulation:
  prologue: qT = Wq.T @ Q.T  [64, 1024]  (4 blocks of 256)
  per strip s (keys [512s, 512s+512)):
    kT_s = Wk.T @ KT_s, vT_s = Wv.T @ VT_s    (PSUM over 8 E-chunks)
    vT_s -> PE-transpose -> v1 rows [512s..] ([k, 65], ones column)
    for q-block j >= s, chunk c in 4s..4s+3:
      e = exp((kT_c.T @ qT_j) / 8) [* causal 0/1 mask if j == s]
      po[j] += v1_c.T @ e          [65, 256] (row 64 = softmax denominator)
    epilogue for block s (po[s] complete):
      out_s = transpose(po[s])[:, :64] * recip(transpose(po[s])[:, 64])
"""
import os
import sys

sys.path.insert(0, "/opt/trn_rl_repo")

import numpy as np

B, N, E, D = 4, 2048, 1024, 64
NQL = N // 2      # local q rows per core
QB = 256          # local q-block width (in qT columns)
KC = 128          # k chunk
EC = 128          # E chunk
NEC = E // EC     # 8
SW = 256          # strip width (keys per strip)
NS = N // SW      # 8 strips
PRECISION = os.environ.get("KERNEL_PRECISION", "f16")

_NC_CACHE = {}


def _build_nc(reps=1):
    from concourse import bacc, mybir, tile
    from concourse.masks import make_identity

    f32 = mybir.dt.float32
    if PRECISION == "f32":
        f32r = mybir.dt.float32
        in_dt = mybir.dt.float32
    elif PRECISION == "f16":
        f32r = mybir.dt.float32r
        in_dt = mybir.dt.float16
    else:
        f32r = mybir.dt.float32r
        in_dt = mybir.dt.float32r
    in_store = mybir.dt.float16 if PRECISION == "f16" else mybir.dt.float32
    av_dt = mybir.dt.float16 if PRECISION == "f16" else f32r
    kq_dt = mybir.dt.float16 if PRECISION == "f16" else f32r
    mask_dt = mybir.dt.float16 if PRECISION == "f16" else mybir.dt.bfloat16
    AF = mybir.ActivationFunctionType

    nc = bacc.Bacc()
    KT = nc.dram_tensor("KT", [NS, EC, NEC, SW], in_store, kind="ExternalInput")
    QT = nc.dram_tensor("QT", [EC, NEC, NQL], in_store, kind="ExternalInput")
    VT = nc.dram_tensor("VT", [NS, EC, NEC, SW], in_store, kind="ExternalInput")
    WK = nc.dram_tensor("WK", [EC, NEC, D], in_store, kind="ExternalInput")
    WQ = nc.dram_tensor("WQ", [EC, NEC, D], in_store, kind="ExternalInput")
    WV = nc.dram_tensor("WV", [EC, NEC, D], in_store, kind="ExternalInput")
    MASK = nc.dram_tensor("MASK", [KC, 4, QB], mask_dt, kind="ExternalInput")
    OUT = nc.dram_tensor("OUT", [NQL // QB, KC, 2, D], f32, kind="ExternalOutput")

    with tile.TileContext(nc) as tc:
        for _rep in range(reps):
            with (
                tc.tile_pool(name=f"consts{_rep}", bufs=1) as consts,
                tc.tile_pool(name=f"qin{_rep}", bufs=2) as qin,
                tc.tile_pool(name=f"kin{_rep}", bufs=2) as kin,
                tc.tile_pool(name=f"vin{_rep}", bufs=2) as vin,
                tc.tile_pool(name=f"proj{_rep}", bufs=1) as proj,
                tc.tile_pool(name=f"expp{_rep}", bufs=6) as expp,
                tc.tile_pool(name=f"epi{_rep}", bufs=2) as epi,
                tc.tile_pool(name=f"psA{_rep}", bufs=1, space="PSUM") as psA,
            ):
                # ---- constants ----
                wk_all = consts.tile([EC, NEC, D], in_dt, tag="wk")
                wq_all = consts.tile([EC, NEC, D], in_dt, tag="wq")
                wv_all = consts.tile([EC, NEC, D], in_dt, tag="wv")
                nc.sync.dma_start(wq_all[:], WQ[:].bitcast(in_dt))
                nc.sync.dma_start(wk_all[:], WK[:].bitcast(in_dt))
                nc.sync.dma_start(wv_all[:], WV[:].bitcast(in_dt))
                masks = consts.tile([KC, 4, QB], mask_dt, tag="mask")
                nc.sync.dma_start(masks[:], MASK[:])
                ident = consts.tile([128, 128], f32, tag="ident")
                make_identity(nc, ident[:])

                kT_sb = proj.tile([D, N], kq_dt, tag="kT")
                qT_sb = proj.tile([D, NQL], kq_dt, tag="qT")
                v1_sb = proj.tile([KC, N // KC, D + 1], av_dt, tag="v1")
                nc.gpsimd.memset(v1_sb[:].bitcast(mybir.dt.float32 if av_dt != mybir.dt.float16 else mybir.dt.float16), 1.0)

                # PSUM pools: po[0..3] (4 banks) + ps/pq2 (2) + pkq (1) + pv/pvt (1)
                po = [psA.tile([D + 1, QB], f32, tag=f"po{j}", name=f"po{j}", bufs=1)
                      for j in range(4)]

                # ---- prologue: q projections (4 blocks of 256) ----
                qt = qin.tile([EC, NEC, NQL], in_dt, tag="qt", bufs=1)
                nc.sync.dma_start(qt[:], QT[:].bitcast(in_dt))
                for j in range(4):
                    pkq = psA.tile([D, SW], f32, tag="pkq", name="pkq", bufs=1)
                    for c in range(NEC):
                        nc.tensor.matmul(pkq[:, 0:QB], wq_all[:, c, :],
                                         qt[:, c, QB * j:QB * (j + 1)],
                                         start=(c == 0), stop=(c == NEC - 1))
                    nc.vector.tensor_copy(qT_sb[:, QB * j:QB * (j + 1)], pkq[:, 0:QB])

                # ---- strips ----
                for s in range(NS):
                    # k projection for keys [SW*s, SW*(s+1))
                    kt = kin.tile([EC, NEC, SW], in_dt, tag="kt")
                    nc.sync.dma_start(kt[:], KT[s].bitcast(in_dt))
                    pkq = psA.tile([D, SW], f32, tag="pkq", name="pkq", bufs=1)
                    for c in range(NEC):
                        nc.tensor.matmul(pkq[:], wk_all[:, c, :], kt[:, c, :],
                                         start=(c == 0), stop=(c == NEC - 1))
                    nc.vector.tensor_copy(kT_sb[:, SW * s:SW * (s + 1)], pkq[:])

                    # v projection + transpose into v1
                    vt = vin.tile([EC, NEC, SW], in_dt, tag="vt")
                    nc.sync.dma_start(vt[:], VT[s].bitcast(in_dt))
                    pv = psA.tile([KC, SW // KC, D], f32, tag="pv", name="pv", bufs=1)
                    for t in range(SW // KC):
                        for c in range(NEC):
                            nc.tensor.matmul(pv[:, t, :], vt[:, c, KC * t:KC * (t + 1)],
                                             wv_all[:, c, :],
                                             start=(c == 0), stop=(c == NEC - 1))
                    for t in range(SW // KC):
                        nc.vector.tensor_copy(v1_sb[:, (SW // KC) * s + t, 0:D], pv[:, t, :])

                    # attention for the new k chunks against q-blocks j >= s//2
                    for j in range((SW * s) // (2 * QB), 4):
                        for m in range(SW // KC):
                            c = (SW // KC) * s + m
                            ps = psA.tile([KC, QB], f32, tag="ps", name="ps", bufs=2)
                            nc.tensor.matmul(ps[:], kT_sb[:, KC * c:KC * (c + 1)],
                                             qT_sb[:, QB * j:QB * (j + 1)], start=True, stop=True)
                            ex = expp.tile([KC, QB], av_dt, tag="ex")
                            nc.scalar.activation(ex[:], ps[:], AF.Exp, scale=0.125)
                            if c >= 4 * j:
                                nc.vector.tensor_mul(ex[:], ex[:], masks[:, c - 4 * j, :])
                            nc.tensor.matmul(po[j][:], v1_sb[:, c, :], ex[:],
                                             start=(c == 0), stop=(c == 4 * j + 3))

                    # epilogue when a q-block just completed (strip covered its last chunks)
                    if (SW * (s + 1)) % (2 * QB) == 0:
                        jj = (SW * (s + 1)) // (2 * QB) - 1
                        pot = epi.tile([D + 1, QB], f32, tag="pot")
                        nc.vector.tensor_copy(pot[:], po[jj][:])
                        ob = epi.tile([KC, 2, D], f32, tag="ob")
                        for h in range(2):
                            pq2 = psA.tile([KC, D + 1], f32, tag="ps", name="pq2", bufs=2)
                            nc.tensor.transpose(pq2[:], pot[:, KC * h:KC * (h + 1)],
                                                ident[0:D + 1, 0:D + 1])
                            rcp = epi.tile([KC, 1], f32, tag="rcp")
                            nc.vector.reciprocal(rcp[:], pq2[:, D:D + 1])
                            nc.vector.tensor_scalar_mul(ob[:, h, :], pq2[:, 0:D], rcp[:])
                        nc.sync.dma_start(OUT[jj], ob[:])

    nc.finalize()
    return nc


def get_nc(reps=1):
    key = ("nc", reps)
    if key not in _NC_CACHE:
        _NC_CACHE[key] = _build_nc(reps)
    return _NC_CACHE[key]


def shard_inputs(K, Q, V, Wk, Wq, Wv):
    in_np = np.float16 if PRECISION == "f16" else np.float32
    K, Q, V = np.asarray(K), np.asarray(Q), np.asarray(V)
    Wkx, Wqx, Wvx = (
        np.ascontiguousarray(np.asarray(a, dtype=np.float32).reshape(NEC, EC, D).transpose(1, 0, 2)).astype(in_np)
        for a in (Wk, Wq, Wv))
    kk = np.arange(KC)
    qq = np.arange(QB)
    masks = {}
    for p in range(2):
        m4 = np.stack([
            (kk[:, None] + KC * m <= 2 * qq[None, :] + p).astype(np.float32)
            for m in range(4)
        ])  # [4, 128, 256]
        if PRECISION == "f16":
            mdt = np.float16
        else:
            import ml_dtypes
            mdt = ml_dtypes.bfloat16
        masks[p] = np.ascontiguousarray(m4.transpose(1, 0, 2).astype(mdt))
    in_maps = []
    for core in range(8):
        b, p = core // 2, core % 2
        kx = np.ascontiguousarray(
            K[b].T.reshape(NEC, EC, NS, SW).transpose(2, 1, 0, 3)).astype(in_np)
        vx = np.ascontiguousarray(
            V[b].T.reshape(NEC, EC, NS, SW).transpose(2, 1, 0, 3)).astype(in_np)
        qx = np.ascontiguousarray(
            Q[b].T[:, p::2].reshape(NEC, EC, NQL).transpose(1, 0, 2)).astype(in_np)
        in_maps.append({
            "KT": kx,
            "QT": qx,
            "VT": vx,
            "WK": Wkx, "WQ": Wqx, "WV": Wvx,
            "MASK": masks[p],
        })
    return in_maps


def gather_outputs(outs):
    full = np.zeros((B, N, D), np.float32)
    for core in range(8):
        b, p = core // 2, core % 2
        o = np.asarray(outs[core])
        if o.ndim == 4:  # [NS, KC, 2, D] -> local rows [NS*2*KC, D]
            o = o.transpose(0, 2, 1, 3).reshape(NQL, D)
        full[b, p::2] = o
    return full


def kernel(K, Q, V, Wk, Wq, Wv):
    from concourse.bass_utils import run_bass_kernel_spmd

    in_maps = shard_inputs(K, Q, V, Wk, Wq, Wv)
    nc = get_nc()
    res = run_bass_kernel_spmd(nc, in_maps, list(range(8)))
    return gather_outputs([res.results[i]["OUT"] for i in range(8)])
